# revision 5
# baseline (speedup 1.0000x reference)
"""Trainium2 Bass kernel for AttnProcessor self-attention (B=2,S=2048,C=1024,H=16).

Sharding: 8 cores, core c owns heads (2c, 2c+1) for both batches (tensor
parallel on the head dim for QKV); attention outputs are redistributed with an
8-core AllToAll so core c computes the output projection + residual for output
rows (b=c//4, s in [512*(c%4), 512*(c%4+1))).

Per-core pipeline (all matmuls fp32r):
  qT/kT = (Wq|Wk slice).T-contracted projections in [c'=128, s] layout,
  v' = [s, 2x(64+ones)] layout (padded to 256 for fp32r full rate),
  scoresT = row-tiled (64x128) QK^T per head pair, exp on ScalarE (scale=1/8,
  no max subtraction -- scores are O(5)), PV accumulates V'.T @ probsT giving
  both the attention output (transposed) and the softmax denominators (ones
  row), normalization happens after the AllToAll, then an 8-chunk contraction
  against full Wo plus (residual + bo) prepared host-side.
"""
import numpy as np

import concourse.bacc as bacc
import concourse.bass as bass
import concourse.tile as tile
from concourse import mybir
from concourse.bass_utils import run_bass_kernel_spmd

F32 = mybir.dt.float32
F32R = mybir.dt.float32r

B, S, C, H, D = 2, 2048, 1024, 16, 64
N_CORES = 8
BS = B * S  # 4096
SCALE = 1.0 / np.sqrt(D)

_CACHE = {}


def _build():
    nc = bacc.Bacc(num_devices=N_CORES)
    hsT = nc.declare_dram_parameter("hsT", [C, BS], F32R, isOutput=False)
    wq = nc.declare_dram_parameter("wq", [C, 128], F32R, isOutput=False)
    wk = nc.declare_dram_parameter("wk", [C, 128], F32R, isOutput=False)
    wv = nc.declare_dram_parameter("wv", [C, 256], F32R, isOutput=False)
    wo = nc.declare_dram_parameter("wo", [C, C], F32R, isOutput=False)
    bqk = nc.declare_dram_parameter("bqk", [128, 2], F32, isOutput=False)
    bvb = nc.declare_dram_parameter("bvb", [1, 256], F32, isOutput=False)
    res = nc.declare_dram_parameter("res", [512, C], F32, isOutput=False)
    out = nc.declare_dram_parameter("out", [512, C], F32, isOutput=True)

    with tile.TileContext(nc) as tc:
        with (
            tc.tile_pool(name="wpool", bufs=1) as wpool,
            tc.tile_pool(name="hpool", bufs=1) as hpool,
            tc.tile_pool(name="qkpool", bufs=2) as qkpool,
            tc.tile_pool(name="ppool", bufs=3) as ppool,
            tc.tile_pool(name="spool", bufs=3) as spool,
            tc.tile_pool(name="opool", bufs=2) as opool,
            tc.tile_pool(name="psum", bufs=1, space="PSUM") as psum,
            tc.tile_pool(name="dram", bufs=1, space="DRAM") as dram,
        ):
            # ---- persistent weights / constants ----
            wq_sb, wk_sb, wv_sb, wo_sb = [], [], [], []
            for cc in range(8):
                t = wpool.tile([128, 128], F32R, tag=f"wq{cc}")
                nc.sync.dma_start(out=t[:], in_=wq[128 * cc:128 * (cc + 1), :])
                wq_sb.append(t)
                t = wpool.tile([128, 128], F32R, tag=f"wk{cc}")
                nc.sync.dma_start(out=t[:], in_=wk[128 * cc:128 * (cc + 1), :])
                wk_sb.append(t)
                t = wpool.tile([128, 256], F32R, tag=f"wv{cc}")
                nc.sync.dma_start(out=t[:], in_=wv[128 * cc:128 * (cc + 1), :])
                wv_sb.append(t)
            bqk_sb = wpool.tile([128, 2], F32, tag="bqk")
            nc.sync.dma_start(out=bqk_sb[:], in_=bqk[:])
            bvb_sb = wpool.tile([128, 256], F32, tag="bvb")
            bvb_ap = bvb[:]
            nc.sync.dma_start(
                out=bvb_sb[:],
                in_=bass.AP(tensor=bvb_ap.tensor, offset=bvb_ap.offset,
                            ap=[[0, 128], [1, 256]]),
            )

            a2a_in = dram.tile([8, 130, 512], F32)
            a2a_out = dram.tile([8, 130, 512], F32)

            qT, kT, vS = {}, {}, {}

            def emit_hsT_load(b):
                tiles = []
                for cc in range(8):
                    t = hpool.tile([128, 2048], F32R, tag=f"hs{cc}")
                    nc.sync.dma_start(
                        out=t[:],
                        in_=hsT[128 * cc:128 * (cc + 1), 2048 * b:2048 * (b + 1)])
                    tiles.append(t)
                return tiles

            def emit_proj_qk(b, hs_sb, t_idx, jpair):
                """One unit: tensor t_idx (0=q,1=k), s-slices jpair, jpair+1."""
                if t_idx == 0:
                    if b not in qT:
                        qT[b] = qkpool.tile([128, 2048], F32R, tag="qT", name=f"qT{b}")
                    dst, w_sb = qT[b], wq_sb
                else:
                    if b not in kT:
                        kT[b] = qkpool.tile([128, 2048], F32R, tag="kT", name=f"kT{b}")
                    dst, w_sb = kT[b], wk_sb
                ps = psum.tile([128, 1024], F32, tag="big", bufs=3)
                for jj in range(2):
                    j = jpair + jj
                    for cc in range(8):
                        nc.tensor.matmul(
                            ps[:, 512 * jj:512 * (jj + 1)],
                            w_sb[cc][:],
                            hs_sb[cc][:, 512 * j:512 * (j + 1)],
                            start=(cc == 0), stop=(cc == 7))
                nc.vector.tensor_scalar_add(
                    out=dst[:, 512 * jpair:512 * (jpair + 2)],
                    in0=ps[:],
                    scalar1=bqk_sb[:, t_idx:t_idx + 1])

            def emit_proj_v(b, hs_sb, ipair):
                """One unit: v' s-tiles ipair, ipair+1 (128 rows each)."""
                if b not in vS:
                    vS[b] = qkpool.tile([128, 4096], F32R, tag="vS", name=f"vS{b}")
                dst = vS[b]
                ps = psum.tile([128, 1024], F32, tag="big", bufs=3)
                for ii in range(2):
                    i = ipair + ii
                    # one bank per s-tile: banks at free offsets 0 and 512
                    sl = ps[:, 512 * ii:512 * ii + 256]
                    for cc in range(8):
                        nc.tensor.matmul(
                            sl,
                            hs_sb[cc][:, 2048 * 0 + 128 * i:128 * (i + 1)],
                            wv_sb[cc][:],
                            start=(cc == 0), stop=(cc == 7))
                # add bias(+ones) broadcast and round to f32r
                nc.vector.tensor_tensor(
                    out=dst[:, 256 * ipair:256 * (ipair + 2)]
                    .rearrange("p (i n) -> p i n", i=2),
                    in0=ps[:].rearrange("p (i n) -> p i n", i=2)[:, :, 0:256],
                    in1=bvb_sb[:, None, :].broadcast_to([128, 2, 256]),
                    op=mybir.AluOpType.add)

            def emit_attention_qs(b, qs, fill_work):
                """One q-slice (256 q) for both heads; fill_work emitted
                mid-loop to soak PE idle while ACT paces."""
                accA = psum.tile([65, 256], F32, tag="acc", bufs=2)
                accB = psum.tile([65, 256], F32, tag="acc", bufs=2)
                for g in range(8):
                    sc = psum.tile([128, 1024], F32, tag="big", bufs=3)
                    pr = ppool.tile([128, 1024], F32R, tag="pr")
                    for half in range(2):
                        kc = 2 * g + half
                        # head A -> bank0 (cols 0:512), head B -> bank1
                        nc.tensor.matmul(
                            sc[:, 256 * half:256 * (half + 1)],
                            kT[b][0:64, 128 * kc:128 * (kc + 1)],
                            qT[b][0:64, 256 * qs:256 * (qs + 1)],
                            start=True, stop=True, tile_position=(0, 0))
                        nc.tensor.matmul(
                            sc[:, 512 + 256 * half:512 + 256 * (half + 1)],
                            kT[b][64:128, 128 * kc:128 * (kc + 1)],
                            qT[b][64:128, 256 * qs:256 * (qs + 1)],
                            start=True, stop=True, tile_position=(64, 0))
                    nc.scalar.activation(pr[:], sc[:],
                                         mybir.ActivationFunctionType.Exp,
                                         scale=float(SCALE))
                    for half in range(2):
                        kc = 2 * g + half
                        nc.tensor.matmul(
                            accA[:],
                            vS[b][:, 256 * kc + 0:256 * kc + 65],
                            pr[:, 256 * half:256 * (half + 1)],
                            start=(g == 0 and half == 0),
                            stop=(g == 7 and half == 1))
                        nc.tensor.matmul(
                            accB[:],
                            vS[b][:, 256 * kc + 65:256 * kc + 130],
                            pr[:, 512 + 256 * half:512 + 256 * (half + 1)],
                            start=(g == 0 and half == 0),
                            stop=(g == 7 and half == 1))
                    if g in (2, 5) and fill_work:
                        fill_work.pop(0)()
                # drain accumulators: [0:64] = attn outT, [64] = softmax sums
                j = 4 * b + qs // 2
                qh = 256 * (qs % 2)
                for h, acc in ((0, accA), (1, accB)):
                    st = spool.tile([65, 256], F32, tag="st")
                    nc.vector.tensor_copy(st[:], acc[:])
                    nc.sync.dma_start(
                        out=a2a_in[j, 64 * h:64 * (h + 1), qh:qh + 256],
                        in_=st[0:64, :])
                    nc.sync.dma_start(
                        out=a2a_in[j, 128 + h:129 + h, qh:qh + 256],
                        in_=st[64:65, :])

            # ---------------- emission ----------------
            hs0 = emit_hsT_load(0)
            for t_idx in range(2):
                for jpair in (0, 2):
                    emit_proj_qk(0, hs0, t_idx, jpair)
            for ipair in range(0, 16, 2):
                emit_proj_v(0, hs0, ipair)

            # batch-1 projection work, interleaved into attention(b0)
            hs1 = emit_hsT_load(1)
            fill = []
            for t_idx in range(2):
                for jpair in (0, 2):
                    fill.append(
                        lambda t=t_idx, jp=jpair: emit_proj_qk(1, hs1, t, jp))
            for ipair in range(0, 16, 2):
                fill.append(lambda ip=ipair: emit_proj_v(1, hs1, ip))

            for qs in range(8):
                emit_attention_qs(0, qs, fill)

            # load wo / res during attention(b1); reuse freed slots
            for cc in range(8):
                t = hpool.tile([128, 1024], F32R, tag=f"hs{cc}",
                               name=f"wo{cc}")
                nc.sync.dma_start(out=t[:], in_=wo[128 * cc:128 * (cc + 1), :])
                wo_sb.append(t)
            res_sb = []
            for st_i in range(4):
                t = qkpool.tile([128, 1024], F32,
                                tag=("qT" if st_i < 2 else "kT"),
                                name=f"res{st_i}")
                nc.sync.dma_start(out=t[:],
                                  in_=res[128 * st_i:128 * (st_i + 1), :])
                res_sb.append(t)

            while fill:
                fill.pop(0)()
            for qs in range(8):
                emit_attention_qs(1, qs, [])

            # ---- AllToAll: redistribute attnT + sums ----
            nc.gpsimd.collective_compute(
                "AllToAll", mybir.AluOpType.bypass,
                replica_groups=[list(range(8))],
                ins=[a2a_in[:]], outs=[a2a_out[:]])

            # ---- normalize received chunks ----
            attn_n = qkpool.tile([128, 4096], F32R, tag="vS", name="attn_n")
            for j in range(8):
                raw = opool.tile([128, 512], F32, tag="raw")
                nc.sync.dma_start(out=raw[:], in_=a2a_out[j, 0:128, :])
                rbc = opool.tile([128, 512], F32, tag="rbc")
                for h in range(2):
                    srow = a2a_out[j, 128 + h:129 + h, :]
                    nc.sync.dma_start(
                        out=rbc[64 * h:64 * (h + 1), :],
                        in_=bass.AP(tensor=srow.tensor, offset=srow.offset,
                                    ap=[[0, 64], [1, 512]]))
                nc.vector.reciprocal(rbc[:], rbc[:])
                nc.vector.tensor_tensor(
                    out=attn_n[:, 512 * j:512 * (j + 1)],
                    in0=raw[:], in1=rbc[:], op=mybir.AluOpType.mult)

            # ---- output projection + residual ----
            for st_i in range(4):
                ps = psum.tile([128, 1024], F32, tag="big", bufs=3)
                for co in range(2):
                    for j in range(8):
                        nc.tensor.matmul(
                            ps[:, 512 * co:512 * (co + 1)],
                            attn_n[:, 512 * j + 128 * st_i:
                                   512 * j + 128 * (st_i + 1)],
                            wo_sb[j][:, 512 * co:512 * (co + 1)],
                            start=(j == 0), stop=(j == 7))
                ob = opool.tile([128, 1024], F32, tag="ob")
                nc.vector.tensor_tensor(out=ob[:], in0=ps[:],
                                        in1=res_sb[st_i][:],
                                        op=mybir.AluOpType.add)
                nc.sync.dma_start(out=out[128 * st_i:128 * (st_i + 1), :],
                                  in_=ob[:])
    nc.finalize()
    return nc


def _prep_inputs(hidden_states, Wq, bq, Wk, bk, Wv, bv, Wo, bo):
    hs = np.asarray(hidden_states, np.float32)
    hsT = np.ascontiguousarray(
        hs.transpose(2, 0, 1).reshape(C, BS)).astype(np.float32)
    Wo_f = np.ascontiguousarray(np.asarray(Wo, np.float32))
    in_maps = []
    for c in range(N_CORES):
        h0 = 2 * c
        cols = slice(64 * h0, 64 * h0 + 128)
        wv_c = np.zeros((C, 256), np.float32)
        bvb_c = np.zeros((1, 256), np.float32)
        for a in range(2):
            hd = slice(64 * (h0 + a), 64 * (h0 + a + 1))
            wv_c[:, 65 * a:65 * a + 64] = Wv[:, hd]
            bvb_c[0, 65 * a:65 * a + 64] = bv[hd]
            bvb_c[0, 65 * a + 64] = 1.0
        bqk_c = np.stack([bq[cols], bk[cols]], axis=1).astype(np.float32)
        b_c, s0 = c // 4, 512 * (c % 4)
        res_c = (hs[b_c, s0:s0 + 512, :] + np.asarray(bo, np.float32)
                 ).astype(np.float32)
        in_maps.append({
            "hsT": hsT,
            "wq": np.ascontiguousarray(Wq[:, cols], np.float32),
            "wk": np.ascontiguousarray(Wk[:, cols], np.float32),
            "wv": wv_c,
            "wo": Wo_f,
            "bqk": bqk_c,
            "bvb": bvb_c,
            "res": np.ascontiguousarray(res_c),
        })
    return in_maps


def _run(inputs, trace=False, trace_kwargs=None):
    if "nc" not in _CACHE:
        _CACHE["nc"] = _build()
    nc = _CACHE["nc"]
    in_maps = _prep_inputs(**inputs)
    r = run_bass_kernel_spmd(nc, in_maps, core_ids=list(range(N_CORES)),
                             trace=trace, **(trace_kwargs or {}))
    full = np.empty((B, S, C), np.float32)
    for c in range(N_CORES):
        full[c // 4, 512 * (c % 4):512 * (c % 4 + 1), :] = r.results[c]["out"]
    return full, r


def kernel(**inputs):
    full, _ = _run(inputs, trace=False)
    return full


# revision 6
# speedup vs baseline: 1.0905x; 1.0905x over previous
"""Trainium2 Bass kernel for AttnProcessor self-attention (B=2,S=2048,C=1024,H=16).

Sharding: 8 cores, core c owns heads (2c, 2c+1) for both batches (tensor
parallel on the head dim for QKV); attention outputs are redistributed with an
8-core AllToAll so core c computes the output projection + residual for output
rows (b=c//4, s in [512*(c%4), 512*(c%4+1))).

Per-core pipeline (all matmuls fp32r):
  qT/kT = (Wq|Wk slice).T-contracted projections in [c'=128, s] layout,
  v' = [s, 2x(64+ones)] layout (padded to 256 for fp32r full rate),
  scoresT = row-tiled (64x128) QK^T per head pair, exp on ScalarE (scale=1/8,
  no max subtraction -- scores are O(5)), PV accumulates V'.T @ probsT giving
  both the attention output (transposed) and the softmax denominators (ones
  row), normalization happens after the AllToAll, then an 8-chunk contraction
  against full Wo plus (residual + bo) prepared host-side.
"""
import numpy as np

import concourse.bacc as bacc
import concourse.bass as bass
import concourse.tile as tile
from concourse import mybir
from concourse.bass_utils import run_bass_kernel_spmd

F32 = mybir.dt.float32
F32R = mybir.dt.float32r

B, S, C, H, D = 2, 2048, 1024, 16, 64
N_CORES = 8
BS = B * S  # 4096
SCALE = 1.0 / np.sqrt(D)

_CACHE = {}


def _build():
    nc = bacc.Bacc(num_devices=N_CORES)
    hsT = nc.declare_dram_parameter("hsT", [C, BS], F32R, isOutput=False)
    wq = nc.declare_dram_parameter("wq", [C, 128], F32R, isOutput=False)
    wk = nc.declare_dram_parameter("wk", [C, 128], F32R, isOutput=False)
    wv = nc.declare_dram_parameter("wv", [C, 256], F32R, isOutput=False)
    wo = nc.declare_dram_parameter("wo", [C, C], F32R, isOutput=False)
    bqk = nc.declare_dram_parameter("bqk", [128, 2], F32, isOutput=False)
    bvb = nc.declare_dram_parameter("bvb", [1, 256], F32, isOutput=False)
    res = nc.declare_dram_parameter("res", [512, C], F32, isOutput=False)
    out = nc.declare_dram_parameter("out", [512, C], F32, isOutput=True)

    with tile.TileContext(nc) as tc:
        with (
            tc.tile_pool(name="wpool", bufs=1) as wpool,
            tc.tile_pool(name="hpool", bufs=1) as hpool,
            tc.tile_pool(name="qkpool", bufs=2) as qkpool,
            tc.tile_pool(name="ppool", bufs=3) as ppool,
            tc.tile_pool(name="spool", bufs=3) as spool,
            tc.tile_pool(name="opool", bufs=2) as opool,
            tc.tile_pool(name="psum", bufs=1, space="PSUM") as psum,
            tc.tile_pool(name="dram", bufs=1, space="DRAM") as dram,
        ):
            # ---- persistent weights / constants ----
            wq_sb, wk_sb, wv_sb, wo_sb = [], [], [], []
            for cc in range(8):
                t = wpool.tile([128, 128], F32R, tag=f"wq{cc}")
                nc.sync.dma_start(out=t[:], in_=wq[128 * cc:128 * (cc + 1), :])
                wq_sb.append(t)
                t = wpool.tile([128, 128], F32R, tag=f"wk{cc}")
                nc.sync.dma_start(out=t[:], in_=wk[128 * cc:128 * (cc + 1), :])
                wk_sb.append(t)
                t = wpool.tile([128, 256], F32R, tag=f"wv{cc}")
                nc.sync.dma_start(out=t[:], in_=wv[128 * cc:128 * (cc + 1), :])
                wv_sb.append(t)
            bqk_sb = wpool.tile([128, 2], F32, tag="bqk")
            nc.sync.dma_start(out=bqk_sb[:], in_=bqk[:])
            bvb_sb = wpool.tile([128, 256], F32, tag="bvb")
            bvb_ap = bvb[:]
            nc.sync.dma_start(
                out=bvb_sb[:],
                in_=bass.AP(tensor=bvb_ap.tensor, offset=bvb_ap.offset,
                            ap=[[0, 128], [1, 256]]),
            )

            a2a_in = dram.tile([8, 130, 512], F32)
            a2a_out = dram.tile([8, 130, 512], F32)

            qT, kT, vS = {}, {}, {}

            def emit_hsT_load(b):
                tiles = []
                for cc in range(8):
                    t = hpool.tile([128, 2048], F32R, tag=f"hs{cc}")
                    nc.sync.dma_start(
                        out=t[:],
                        in_=hsT[128 * cc:128 * (cc + 1), 2048 * b:2048 * (b + 1)])
                    tiles.append(t)
                return tiles

            def emit_proj_qk(b, hs_sb, t_idx, jpair):
                """One unit: tensor t_idx (0=q,1=k), s-slices jpair, jpair+1."""
                if t_idx == 0:
                    if b not in qT:
                        qT[b] = qkpool.tile([128, 2048], F32R, tag="qT", name=f"qT{b}")
                    dst, w_sb = qT[b], wq_sb
                else:
                    if b not in kT:
                        kT[b] = qkpool.tile([128, 2048], F32R, tag="kT", name=f"kT{b}")
                    dst, w_sb = kT[b], wk_sb
                ps = psum.tile([128, 1024], F32, tag="big", bufs=3)
                for jj in range(2):
                    j = jpair + jj
                    for cc in range(8):
                        nc.tensor.matmul(
                            ps[:, 512 * jj:512 * (jj + 1)],
                            w_sb[cc][:],
                            hs_sb[cc][:, 512 * j:512 * (j + 1)],
                            start=(cc == 0), stop=(cc == 7))
                nc.vector.tensor_scalar_add(
                    out=dst[:, 512 * jpair:512 * (jpair + 2)],
                    in0=ps[:],
                    scalar1=bqk_sb[:, t_idx:t_idx + 1])

            def emit_proj_v(b, hs_sb, ipair):
                """One unit: v' s-tiles ipair, ipair+1 (128 rows each)."""
                if b not in vS:
                    vS[b] = qkpool.tile([128, 4096], F32R, tag="vS", name=f"vS{b}")
                dst = vS[b]
                ps = psum.tile([128, 1024], F32, tag="big", bufs=3)
                for ii in range(2):
                    i = ipair + ii
                    # one bank per s-tile: banks at free offsets 0 and 512
                    sl = ps[:, 512 * ii:512 * ii + 256]
                    for cc in range(8):
                        nc.tensor.matmul(
                            sl,
                            hs_sb[cc][:, 2048 * 0 + 128 * i:128 * (i + 1)],
                            wv_sb[cc][:],
                            start=(cc == 0), stop=(cc == 7))
                # add bias(+ones) broadcast and round to f32r
                nc.vector.tensor_tensor(
                    out=dst[:, 256 * ipair:256 * (ipair + 2)]
                    .rearrange("p (i n) -> p i n", i=2),
                    in0=ps[:].rearrange("p (i n) -> p i n", i=2)[:, :, 0:256],
                    in1=bvb_sb[:, None, :].broadcast_to([128, 2, 256]),
                    op=mybir.AluOpType.add)

            def emit_attention_qs(b, qs, fill_work):
                """One q-slice (512 q) for both heads, software-pipelined:
                QK(kc+1) is emitted before PV(kc) so ACT paces the loop."""
                accA = psum.tile([65, 512], F32, tag="accA", bufs=1,
                                 name=f"accA_{b}_{qs}")
                accB = psum.tile([65, 512], F32, tag="accB", bufs=1,
                                 name=f"accB_{b}_{qs}")
                sc_t, pr_t = {}, {}

                def emit_qk(kc):
                    sc = psum.tile([128, 1024], F32, tag="big", bufs=3,
                                   name=f"sc_{b}_{qs}_{kc}")
                    sc_t[kc] = sc
                    nc.tensor.matmul(
                        sc[:, 0:512],
                        kT[b][0:64, 128 * kc:128 * (kc + 1)],
                        qT[b][0:64, 512 * qs:512 * (qs + 1)],
                        start=True, stop=True, tile_position=(0, 0))
                    nc.tensor.matmul(
                        sc[:, 512:1024],
                        kT[b][64:128, 128 * kc:128 * (kc + 1)],
                        qT[b][64:128, 512 * qs:512 * (qs + 1)],
                        start=True, stop=True, tile_position=(64, 0))

                emit_qk(0)
                for kc in range(16):
                    pr = ppool.tile([128, 1024], F32R, tag="pr",
                                    name=f"pr_{b}_{qs}_{kc}")
                    nc.scalar.activation(pr[:], sc_t.pop(kc)[:],
                                         mybir.ActivationFunctionType.Exp,
                                         scale=float(SCALE))
                    if kc < 15:
                        emit_qk(kc + 1)
                    nc.tensor.matmul(
                        accA[:],
                        vS[b][:, 256 * kc + 0:256 * kc + 65],
                        pr[:, 0:512],
                        start=(kc == 0), stop=(kc == 15))
                    nc.tensor.matmul(
                        accB[:],
                        vS[b][:, 256 * kc + 65:256 * kc + 130],
                        pr[:, 512:1024],
                        start=(kc == 0), stop=(kc == 15))
                    if kc in (4, 9, 14) and fill_work:
                        fill_work.pop(0)()
                # drain accumulators: [0:64] = attn outT, [64] = softmax sums
                j = 4 * b + qs
                for h, acc in ((0, accA), (1, accB)):
                    st = spool.tile([65, 512], F32, tag="st",
                                    name=f"st_{b}_{qs}_{h}")
                    nc.vector.tensor_copy(st[:], acc[:])
                    nc.sync.dma_start(
                        out=a2a_in[j, 64 * h:64 * (h + 1), :],
                        in_=st[0:64, :])
                    nc.sync.dma_start(
                        out=a2a_in[j, 128 + h:129 + h, :],
                        in_=st[64:65, :])

            # ---------------- emission ----------------
            hs0 = emit_hsT_load(0)
            for t_idx in range(2):
                for jpair in (0, 2):
                    emit_proj_qk(0, hs0, t_idx, jpair)
            for ipair in range(0, 16, 2):
                emit_proj_v(0, hs0, ipair)

            # batch-1 projection work, interleaved into attention(b0)
            hs1 = emit_hsT_load(1)
            fill = []
            for t_idx in range(2):
                for jpair in (0, 2):
                    fill.append(
                        lambda t=t_idx, jp=jpair: emit_proj_qk(1, hs1, t, jp))
            for ipair in range(0, 16, 2):
                fill.append(lambda ip=ipair: emit_proj_v(1, hs1, ip))

            for qs in range(4):
                emit_attention_qs(0, qs, fill)

            # load wo / res during attention(b1); reuse freed slots
            for cc in range(8):
                t = hpool.tile([128, 1024], F32R, tag=f"hs{cc}",
                               name=f"wo{cc}")
                nc.sync.dma_start(out=t[:], in_=wo[128 * cc:128 * (cc + 1), :])
                wo_sb.append(t)
            res_sb = []
            for st_i in range(4):
                t = qkpool.tile([128, 1024], F32,
                                tag=("qT" if st_i < 2 else "kT"),
                                name=f"res{st_i}")
                nc.sync.dma_start(out=t[:],
                                  in_=res[128 * st_i:128 * (st_i + 1), :])
                res_sb.append(t)

            while fill:
                fill.pop(0)()
            for qs in range(4):
                emit_attention_qs(1, qs, [])

            # ---- AllToAll: redistribute attnT + sums ----
            nc.gpsimd.collective_compute(
                "AllToAll", mybir.AluOpType.bypass,
                replica_groups=[list(range(8))],
                ins=[a2a_in[:]], outs=[a2a_out[:]])

            # ---- batched reciprocal of all softmax sums ----
            sums_sb = opool.tile([16, 512], F32, tag="sums", bufs=1)
            for j in range(8):
                nc.sync.dma_start(out=sums_sb[2 * j:2 * j + 2, :],
                                  in_=a2a_out[j, 128:130, :])
            nc.vector.reciprocal(sums_sb[:], sums_sb[:])
            rec_d = dram.tile([16, 512], F32)
            nc.sync.dma_start(out=rec_d[:], in_=sums_sb[:])

            # ---- normalize chunks and run outproj per chunk ----
            op_ps = []
            for st_i in range(4):
                if st_i < 3:
                    ps = psum.tile([128, 1024], F32, tag="big", bufs=3,
                                   name=f"op{st_i}")
                    op_ps.append((ps[:, 0:512], ps[:, 512:1024], ps))
                else:
                    pa = psum.tile([128, 512], F32, tag="accA", bufs=1,
                                   name="op3a")
                    pb = psum.tile([128, 512], F32, tag="accB", bufs=1,
                                   name="op3b")
                    op_ps.append((pa[:], pb[:], None))
            attn_n = qkpool.tile([128, 4096], F32R, tag="vS", name="attn_n")
            for j in range(8):
                raw = opool.tile([128, 512], F32, tag="raw",
                                 name=f"raw{j}")
                nc.sync.dma_start(out=raw[:], in_=a2a_out[j, 0:128, :])
                rbc = opool.tile([128, 512], F32, tag="rbc",
                                 name=f"rbc{j}")
                for h in range(2):
                    srow = rec_d[2 * j + h:2 * j + h + 1, :]
                    nc.sync.dma_start(
                        out=rbc[64 * h:64 * (h + 1), :],
                        in_=bass.AP(tensor=srow.tensor, offset=srow.offset,
                                    ap=[[0, 64], [1, 512]]))
                an = attn_n[:, 512 * j:512 * (j + 1)]
                nc.vector.tensor_tensor(out=an, in0=raw[:], in1=rbc[:],
                                        op=mybir.AluOpType.mult)
                for st_i in range(4):
                    for co in range(2):
                        nc.tensor.matmul(
                            op_ps[st_i][co],
                            an[:, 128 * st_i:128 * (st_i + 1)],
                            wo_sb[j][:, 512 * co:512 * (co + 1)],
                            start=(j == 0), stop=(j == 7))

            # ---- residual add + store ----
            for st_i in range(4):
                ob = opool.tile([128, 1024], F32, tag="ob", name=f"ob{st_i}")
                if st_i < 3:
                    nc.vector.tensor_tensor(out=ob[:], in0=op_ps[st_i][2][:],
                                            in1=res_sb[st_i][:],
                                            op=mybir.AluOpType.add)
                else:
                    for co in range(2):
                        nc.vector.tensor_tensor(
                            out=ob[:, 512 * co:512 * (co + 1)],
                            in0=op_ps[st_i][co],
                            in1=res_sb[st_i][:, 512 * co:512 * (co + 1)],
                            op=mybir.AluOpType.add)
                nc.sync.dma_start(out=out[128 * st_i:128 * (st_i + 1), :],
                                  in_=ob[:])
    nc.finalize()
    return nc


def _prep_inputs(hidden_states, Wq, bq, Wk, bk, Wv, bv, Wo, bo):
    hs = np.asarray(hidden_states, np.float32)
    hsT = np.ascontiguousarray(
        hs.transpose(2, 0, 1).reshape(C, BS)).astype(np.float32)
    Wo_f = np.ascontiguousarray(np.asarray(Wo, np.float32))
    in_maps = []
    for c in range(N_CORES):
        h0 = 2 * c
        cols = slice(64 * h0, 64 * h0 + 128)
        wv_c = np.zeros((C, 256), np.float32)
        bvb_c = np.zeros((1, 256), np.float32)
        for a in range(2):
            hd = slice(64 * (h0 + a), 64 * (h0 + a + 1))
            wv_c[:, 65 * a:65 * a + 64] = Wv[:, hd]
            bvb_c[0, 65 * a:65 * a + 64] = bv[hd]
            bvb_c[0, 65 * a + 64] = 1.0
        bqk_c = np.stack([bq[cols], bk[cols]], axis=1).astype(np.float32)
        b_c, s0 = c // 4, 512 * (c % 4)
        res_c = (hs[b_c, s0:s0 + 512, :] + np.asarray(bo, np.float32)
                 ).astype(np.float32)
        in_maps.append({
            "hsT": hsT,
            "wq": np.ascontiguousarray(Wq[:, cols], np.float32),
            "wk": np.ascontiguousarray(Wk[:, cols], np.float32),
            "wv": wv_c,
            "wo": Wo_f,
            "bqk": bqk_c,
            "bvb": bvb_c,
            "res": np.ascontiguousarray(res_c),
        })
    return in_maps


def _run(inputs, trace=False, trace_kwargs=None):
    if "nc" not in _CACHE:
        _CACHE["nc"] = _build()
    nc = _CACHE["nc"]
    in_maps = _prep_inputs(**inputs)
    r = run_bass_kernel_spmd(nc, in_maps, core_ids=list(range(N_CORES)),
                             trace=trace, **(trace_kwargs or {}))
    full = np.empty((B, S, C), np.float32)
    for c in range(N_CORES):
        full[c // 4, 512 * (c % 4):512 * (c % 4 + 1), :] = r.results[c]["out"]
    return full, r


def kernel(**inputs):
    full, _ = _run(inputs, trace=False)
    return full
